# revision 33
# baseline (speedup 1.0000x reference)
"""Trainium2 Bass kernel for a 12-layer attention-only decoder.

Model (see harness reference): S=24, B=256, D=1024, H=16 heads (dh=64),
L=12 layers, V=32000 vocab.  Per layer: q/k/v projections, softmax
attention (scale 1/sqrt(D), no mask applied), residual add.  Final vocab
head x @ out_w.T + out_b.

Sharding: data-parallel over batch - 8 cores x 32 batches each.
Embedding gather + positional-encoding add is done on host (0.006% of
FLOPs); everything else runs on device in bf16 matmuls with fp32
accumulation.

On-device layout (per core, 768 tokens = 32 batches x 24 positions):
  - residual x kept feature-major: xT[d, t] as SBUF [128, 8, 768]
  - q/k projections produce feature-major qT/kT (stationary = w^T chunk)
  - v projection produces token-major v (stationary = xT chunk), padded
    so each batch sits at a 32-aligned partition base (24 rows used + 8
    pad) -> attention matmul operands satisfy the 32/64-alignment rules
  - scores^T[s', s] = matmul(lhsT=kT slice [64,24], rhs=qT slice [64,24])
  - exp via ScalarE (scale 1/32 folded into q), Z via ones-matmul,
    normalize on VectorE, AV: matmul(lhsT=v slice [24,64], rhs=attn^T
    [24,24]) -> o^T feature-major, accumulated straight into xT
  - vocab head token-major: psum [128 tokens, 512 vocab] tiles, bias add
    on VectorE, DMA straight out.
"""

import math

import numpy as np
import ml_dtypes

import concourse.bass as bass
import concourse.mybir as mybir
import concourse.tile as tile
from concourse import bacc
from concourse.bass_utils import run_bass_kernel_spmd

S, B, D, H, L, V = 24, 256, 1024, 16, 12, 32000
DH = D // H  # 64
NCORES = 8
BL = B // NCORES          # 32 local batches
T = BL * S                # 768 local tokens
KO = D // 128             # 8 contraction chunks
SCALE = 1.0 / math.sqrt(D)

F32 = mybir.dt.float32
BF16 = mybir.dt.bfloat16
F8 = mybir.dt.float8e4
WSCALE = 64.0   # weight pre-scale so fp8(e4m3) quantization of 0.02-scale
                # weights stays in the normal range (min normal 2^-6)

_CACHE = {}


def _build_kernel(n_layers=L, do_attn=True, do_head=True, use_vb=False,
                  use_qkb=True):
    nc = bacc.Bacc(None, target_bir_lowering=False)

    # All inputs pre-arranged on host so every DMA is contiguous per
    # partition (no strided descriptor floods).
    x0t_d = nc.dram_tensor("x0t", [128, KO, T], F32, kind="ExternalInput")
    # q/k weights in fp8 (pre-scaled by WSCALE on host): the q/k path error is
    # suppressed by softmax averaging, so fp8 DoubleRow is accuracy-safe here
    # (unlike the v/head path, which stays bf16)
    qwt_d = nc.dram_tensor("qwt", [L, 128, KO, D], F8, kind="ExternalInput")
    kwt_d = nc.dram_tensor("kwt", [L, 128, KO, D], F8, kind="ExternalInput")
    vwt_d = nc.dram_tensor("vwt", [L, 128, KO, D], BF16, kind="ExternalInput")
    qbs_d = nc.dram_tensor("qbs", [128, L, KO], F32, kind="ExternalInput")
    kb_d = nc.dram_tensor("kb", [128, L, KO], F32, kind="ExternalInput")
    if use_vb:
        vbb_d = nc.dram_tensor("vbb", [L, D], BF16, kind="ExternalInput")
    owt_d = nc.dram_tensor("owt", [128, KO, V], BF16, kind="ExternalInput")
    out_d = nc.dram_tensor("out", [T, V], F32, kind="ExternalOutput")

    Ident = mybir.ActivationFunctionType.Identity
    Exp = mybir.ActivationFunctionType.Exp
    Add = mybir.AluOpType.add
    Mult = mybir.AluOpType.mult

    with tile.TileContext(nc) as tc:
        # zero all PSUM once: stale device PSUM may hold inf/NaN, which would
        # poison the block-diag Z matmul via 0*inf
        with tc.tile_pool(name="psinit", bufs=1, space="PSUM") as psi:
            for i in range(8):
                zb = psi.tile([128, 512], F32, name=f"zb_{i}", tag=f"zb_{i}")
                nc.vector.memset(zb[:], 0.0)

        with (
            tc.tile_pool(name="persist", bufs=1) as persist,
            tc.tile_pool(name="psA", bufs=4, space="PSUM") as psA,   # proj/head [128,512]
        ):
            # ---- persistent SBUF state ----
            xt = persist.tile([128, KO, T], F32)        # residual, feature-major
            xbf = persist.tile([128, KO, T], BF16)      # bf16 copy for matmuls
            xf8 = persist.tile([128, KO, T], F8)        # fp8 copy for q/k proj
            qb_sb = persist.tile([128, L, KO], F32)
            kb_sb = persist.tile([128, L, KO], F32)
            if use_vb:
                ones_sb = persist.tile([128, 128], BF16)
            onesblk = persist.tile([128, 120], BF16)  # block-diag ones for Z

            # x0 arrives per-ko so the first projection matmuls can start
            # as soon as chunk 0 + its weight chunk land (instead of after
            # the full 3.1MB transfer)
            for ko in range(KO):
                # scalar-engine queue: parallel to the weight DMAs issuing
                # from Sync (~700ns per issue), so chunk 0 lands sooner
                nc.scalar.dma_start(xt[:, ko, :], x0t_d[:, ko, :])
            nc.sync.dma_start(qb_sb[:], qbs_d[:])
            nc.sync.dma_start(kb_sb[:], kb_d[:])
            if use_vb:
                nc.vector.memset(ones_sb[:], 1.0)
            nc.vector.memset(onesblk[:], 0.0)
            for bi in range(4):
                nc.vector.memset(onesblk[bi * 32:bi * 32 + S, bi * 32:bi * 32 + S], 1.0)

            def recast():
                # xbf/xf8 <- xt, chunked so deps resolve per-ko
                for ko in range(KO):
                    nc.vector.tensor_copy(xbf[:, ko, :], xt[:, ko, :])
                    nc.vector.tensor_copy(xf8[:, ko, :], xt[:, ko, :])

            # ================= layers =================
            with (
                tc.tile_pool(name="wpool", bufs=2) as wpool,
                tc.tile_pool(name="acts", bufs=1) as acts,
                tc.tile_pool(name="epool", bufs=9) as epool,
                tc.tile_pool(name="rzpool", bufs=2) as rzpool,
                tc.tile_pool(name="psB", bufs=4, space="PSUM") as psB,  # scores/Z/oT [128,384]
            ):
                qt = acts.tile([128, KO, T], BF16, tag="qt")
                kt = acts.tile([128, KO, T], BF16, tag="kt")
                vt = acts.tile([128, 8, D], BF16, tag="vtok")  # [part, bg, o] token-major
                vtmp = acts.tile([128, 6, D], BF16, tag="vtmp")  # unpadded token-major v

                recast()
                for l in range(n_layers):
                    qw_t = wpool.tile([128, KO, D], F8, tag="qw")
                    kw_t = wpool.tile([128, KO, D], F8, tag="kw")
                    vw_t = wpool.tile([128, KO, D], BF16, tag="vw")
                    if l == 0:
                        # per-ki so the first matmul only waits on chunk 0
                        for ki in range(KO):
                            nc.sync.dma_start(qw_t[:, ki, :], qwt_d[l][:, ki, :])
                    else:
                        nc.sync.dma_start(qw_t[:], qwt_d[l])
                    nc.sync.dma_start(kw_t[:], kwt_d[l])
                    nc.sync.dma_start(vw_t[:], vwt_d[l])
                    if use_vb:
                        vbl = wpool.tile([1, D], BF16, tag="vbl", name=f"vbl_{l}")
                        nc.sync.dma_start(vbl[:], vbb_d[l][None, :])

                    # ---- Q, K projections (feature-major out), fp8 DoubleRow:
                    # each matmul contracts 2 ki-planes (256 inputs), halving
                    # the PE instruction/LDW stream vs bf16 ----
                    DR = mybir.MatmulPerfMode.DoubleRow
                    # oi-outer with Q/K interleaved: consecutive psA groups
                    # retire on alternating engines (ScalarE for Q, DVE for K),
                    # so the 4-deep psum ring never drains against one queue
                    for oi in range(8):
                        for w_t, b_sb, dst, sc, nm in (
                            (qw_t, qb_sb, qt, SCALE / WSCALE, "q"),
                            (kw_t, kb_sb, kt, 1.0 / WSCALE, "k"),
                        ):
                            bias_ap = b_sb[:, l, oi:oi + 1]
                            for t0 in (0, 384):
                                ps = psA.tile([128, 512], F32, tag="proj",
                                              name=f"p_{l}_{nm}_{oi}_{t0}")
                                for kd in range(KO // 2):
                                    lhsT = w_t[:, 2 * kd:2 * kd + 2, oi * 128:(oi + 1) * 128]
                                    nc.tensor.matmul(ps[:, 0:384], lhsT,
                                                     xf8[:, 2 * kd:2 * kd + 2, t0:t0 + 384],
                                                     start=(kd == 0), stop=(kd == KO // 2 - 1),
                                                     perf_mode=DR)
                                if use_qkb:
                                    nc.scalar.activation(dst[:, oi, t0:t0 + 384], ps[:, 0:384],
                                                         Ident, bias=bias_ap, scale=sc)
                                elif nm == "k":
                                    # zero-bias K: scaled cast on DVE so the
                                    # (now DR-fast) PE isn't throttled by the
                                    # ScalarE activation stream via psA reuse
                                    nc.vector.tensor_scalar_mul(
                                        dst[:, oi, t0:t0 + 384], ps[:, 0:384], sc)
                                else:
                                    nc.scalar.activation(dst[:, oi, t0:t0 + 384],
                                                         ps[:, 0:384], Ident, scale=sc)

                    # ---- V projection (token-major, unpadded: 6 tiles of 128
                    # real tokens instead of 8 padded groups; saves 25% of the
                    # V matmuls).  A DMA then remaps into the 32-aligned padded
                    # layout vt that attention's tile_position rules need. ----
                    for tt in range(6):
                        pv0 = psA.tile([128, 512], F32, tag="proj", name=f"pv0_{l}_{tt}")
                        pv1 = psA.tile([128, 512], F32, tag="proj", name=f"pv1_{l}_{tt}")
                        for ki in range(KO):
                            lhsT = xbf[:, ki, tt * 128:(tt + 1) * 128]
                            last = (ki == KO - 1) and not use_vb
                            nc.tensor.matmul(pv0[:], lhsT, vw_t[:, ki, 0:512],
                                             start=(ki == 0), stop=last)
                            nc.tensor.matmul(pv1[:], lhsT, vw_t[:, ki, 512:1024],
                                             start=(ki == 0), stop=last)
                        if use_vb:
                            # + bias via rank-1 ones (x) vb, accumulated into psum
                            nc.tensor.matmul(pv0[:], ones_sb[0:1, :], vbl[0:1, 0:512],
                                             start=False, stop=True)
                            nc.tensor.matmul(pv1[:], ones_sb[0:1, :], vbl[0:1, 512:1024],
                                             start=False, stop=True)
                        nc.vector.tensor_copy(vtmp[:, tt, 0:512], pv0[:])
                        nc.vector.tensor_copy(vtmp[:, tt, 512:1024], pv1[:])
                    # remap: batch b tokens (b*24 .. b*24+24) -> vt rows
                    # (b%4)*32.. of group b//4 (SBUF->SBUF DMA, off PE path)
                    for b in range(BL):
                        t0 = b * S
                        tt, p0 = divmod(t0, 128)
                        bg, bi = b // 4, b % 4
                        n1 = min(128 - p0, S)
                        nc.sync.dma_start(vt[bi * 32:bi * 32 + n1, bg, :],
                                          vtmp[p0:p0 + n1, tt, :])
                        if n1 < S:
                            nc.sync.dma_start(vt[bi * 32 + n1:bi * 32 + S, bg, :],
                                              vtmp[0:S - n1, tt + 1, :])

                    # ---- attention ----
                    # exp_t column layout: col(h) = (h%2)*192 + (h//2)*24
                    alv = 4 if do_attn is True else float(do_attn)
                    exp_ts = []

                    def z_norm(bg):
                        # Z: one block-diag ones matmul -> Z replicated per row
                        exp_t = exp_ts[bg]
                        z_ps = psB.tile([128, 384], F32, tag="p384", name=f"z_{l}_{bg}")
                        nc.tensor.matmul(
                            z_ps[0:120, :], onesblk[0:120, :], exp_t[0:120, :],
                            start=True, stop=True, tile_position=(0, 0))
                        rz = rzpool.tile([128, 384], F32, tag="rz", name=f"rz_{l}_{bg}")
                        # ~18 correct bits, 5x faster than reciprocal(); Z>=24
                        # is well inside its safe range (attn is bf16 anyway)
                        nc.vector.reciprocal_approx_fast(rz[0:120, :], z_ps[0:120, :])
                        if alv < 3:
                            return
                        nc.vector.tensor_tensor(exp_t[0:120, :], exp_t[0:120, :],
                                                rz[0:120, :], Mult)

                    for bg in range(8 if alv >= 1 else 0):
                        # scores^T: even heads (kt/qt rows 0:64) -> row-group-0
                        # bank; odd heads (rows 64:128) -> row-group-64 bank.
                        sc_e = psB.tile([128, 192], F32, tag="p384", name=f"se_{l}_{bg}")
                        sc_o = psB.tile([128, 192], F32, tag="p384", name=f"so_{l}_{bg}")
                        for bi in range(4):
                            b = bg * 4 + bi
                            tcol = b * S
                            for hj in range(8):
                                for par, sc_ps in ((0, sc_e), (1, sc_o)):
                                    pb = par * 64
                                    nc.tensor.matmul(
                                        sc_ps[bi * 32:bi * 32 + S, hj * S:(hj + 1) * S],
                                        kt[pb:pb + DH, hj, tcol:tcol + S],
                                        qt[pb:pb + DH, hj, tcol:tcol + S],
                                        start=True, stop=True,
                                        tile_position=(pb, bi * 32))
                        # exp_t interleaved: head h=2j -> cols j*48, h=2j+1 ->
                        # cols j*48+24, so a head-pair is a contiguous 48-col
                        # block (lets AV pair 2 heads per matmul)
                        exp_t = epool.tile([128, 384], BF16, tag="expt", name=f"ex_{l}_{bg}")
                        exp_ts.append(exp_t)
                        e4 = exp_t[:].rearrange("p (j two s) -> p j two s", two=2, s=S)
                        nc.scalar.activation(e4[:, :, 0, :], sc_e[:].rearrange(
                            "p (j s) -> p j s", s=S), Exp)
                        nc.scalar.activation(e4[:, :, 1, :], sc_o[:].rearrange(
                            "p (j s) -> p j s", s=S), Exp)
                        # Z(bg-3) interleaved 3 groups behind: at lag 2 the Z
                        # matmul still waited ~250ns on ScalarE's exp; lag 3
                        # gives ~2.3us of slack so the PE never stalls.
                        if alv >= 2 and bg >= 3:
                            z_norm(bg - 3)
                    for bg in range(5, 8) if alv >= 2 else ():
                        z_norm(bg)

                    # AV: bank = (head pair hp, batch-slot class bi); the 16
                    # matmuls in a bank share row group bi*32; cols g*24.
                    for hp in range(8 if alv >= 4 else 0):
                        for bi in range(4):
                            o_ps = psB.tile([128, 384], F32, tag="p384", name=f"o_{l}_{hp}_{bi}")
                            for g in range(8):
                                for hh in range(2):
                                    nc.tensor.matmul(
                                        o_ps[hh * 64:hh * 64 + DH, g * S:(g + 1) * S],
                                        vt[bi * 32:bi * 32 + S, g,
                                           (hp * 2 + hh) * DH:(hp * 2 + hh + 1) * DH],
                                        exp_ts[g][bi * 32:bi * 32 + S,
                                                  hp * 48 + hh * S:hp * 48 + (hh + 1) * S],
                                        start=True, stop=True,
                                        tile_position=(bi * 32, hh * 64))
                            # residual: b = g*4+bi -> xt cols g*96 + bi*24
                            xsl = xt[:, hp, :].rearrange(
                                "p (g f) -> p g f", f=96)[:, :, bi * S:(bi + 1) * S]
                            nc.vector.tensor_tensor(
                                xsl, xsl,
                                o_ps[:, 0:192].rearrange("p (g f) -> p g f", f=S), Add)
                        # head-pair residuals done for all batches: refresh the
                        # bf16/fp8 copies of xT chunk ko=hp on ScalarE (DVE is
                        # busy with the residual adds)
                        nc.scalar.copy(xbf[:, hp, :], xt[:, hp, :])
                        nc.scalar.copy(xf8[:, hp, :], xt[:, hp, :])

            # ================= vocab head =================
            if n_layers == 0 or (do_attn is not True and float(do_attn) < 4):
                recast()
            CHUNK = 2048
            with (
                tc.tile_pool(name="owpool", bufs=3) as owpool,
                tc.tile_pool(name="lgpool", bufs=8) as lgpool,
                tc.tile_pool(name="psH", bufs=4, space="PSUM") as psH,
            ):
                # first chunks are small so the head's first matmuls only
                # wait on a ~1MB out_w DMA instead of 4MB
                chunks = []
                c0 = 0
                for cw in (512, 1536):
                    if do_head:
                        chunks.append((c0, cw)); c0 += cw
                while do_head and c0 < V:
                    chunks.append((c0, min(CHUNK, V - c0))); c0 += CHUNK
                for c0, cw in chunks:
                    owc = owpool.tile([128, KO, CHUNK], BF16, tag="ow", name=f"ow_{c0}")
                    for ki in range(KO):
                        nc.sync.dma_start(
                            owc[:, ki, 0:cw], owt_d[:, ki, c0:c0 + cw])
                    for sub0 in range(0, cw, 512):
                        sw = min(512, cw - sub0)
                        for tt in range(T // 128):
                            ps = psH.tile([128, 512], F32, tag="hps", name=f"h_{c0}_{sub0}_{tt}")
                            for ki in range(KO):
                                nc.tensor.matmul(
                                    ps[:, 0:sw],
                                    xbf[:, ki, tt * 128:(tt + 1) * 128],
                                    owc[:, ki, sub0:sub0 + sw],
                                    start=(ki == 0), stop=(ki == KO - 1))
                            lg = lgpool.tile([128, 512], F32, tag="lg", name=f"lg_{c0}_{sub0}_{tt}")
                            nc.vector.tensor_copy(lg[:, 0:sw], ps[:, 0:sw])
                            nc.sync.dma_start(
                                out_d[tt * 128:(tt + 1) * 128, c0 + sub0:c0 + sub0 + sw],
                                lg[:, 0:sw])
    nc.finalize()
    return nc


def kernel(**inputs):
    inputs = {k: np.asarray(v) for k, v in inputs.items()}
    tok = inputs["inputs"]            # [S, B] int
    emb = inputs["token_embed"]       # [V, D] f32
    pe = inputs["pe"]                 # [27, 1, D] f32
    qw, qb = inputs["qw"], inputs["qb"]
    kw, kb = inputs["kw"], inputs["kb"]
    vw, vb = inputs["vw"], inputs["vb"]
    out_w, out_b = inputs["out_w"], inputs["out_b"]

    s_len = tok.shape[0]
    x0 = emb[tok] + pe[:s_len]        # [S, B, D] f32 (host: 0.006% of FLOPs)
    x0 = np.ascontiguousarray(x0, dtype=np.float32)

    bf = ml_dtypes.bfloat16
    # pre-arranged for contiguous per-partition DMA: w^T [L, D, D] ->
    # [L, 128, KO, D] with in-feature d = ko*128 + p
    def warr(w, dt=bf, scale=1.0):
        wt = np.ascontiguousarray(w.transpose(0, 2, 1) * scale).astype(dt)
        return np.ascontiguousarray(wt.reshape(L, KO, 128, D).transpose(0, 2, 1, 3))
    f8 = ml_dtypes.float8_e4m3fn
    qwt = warr(qw, f8, WSCALE)
    kwt = warr(kw, f8, WSCALE)
    vwt = warr(vw)
    owt = np.ascontiguousarray(out_w.T).astype(bf)          # [D, V]
    owt = np.ascontiguousarray(owt.reshape(KO, 128, V).transpose(1, 0, 2))
    qbs = (qb.astype(np.float32) * SCALE).reshape(L, KO, 128).transpose(2, 0, 1)
    qbs = np.ascontiguousarray(qbs)
    kbf = np.ascontiguousarray(kb.astype(np.float32).reshape(L, KO, 128).transpose(2, 0, 1))
    use_vb = bool(np.any(vb))
    use_qkb = bool(np.any(qb) or np.any(kb))

    key = (use_vb, use_qkb)
    if _CACHE.get("flags") != key:
        _CACHE["nc"] = _build_kernel(use_vb=use_vb, use_qkb=use_qkb)
        _CACHE["flags"] = key
    nc = _CACHE["nc"]

    shared = {
        "qwt": qwt, "kwt": kwt, "vwt": vwt,
        "qbs": qbs, "kb": kbf,
        "owt": owt,
    }
    if use_vb:
        shared["vbb"] = vb.astype(bf)
    in_maps = []
    for c in range(NCORES):
        xc = x0[:, c * BL:(c + 1) * BL, :]            # [S, BL, D]
        x0t = np.ascontiguousarray(xc.transpose(2, 1, 0).reshape(D, T))
        x0r = np.ascontiguousarray(
            x0t.reshape(KO, 128, T).transpose(1, 0, 2), dtype=np.float32)
        in_maps.append({"x0t": x0r, **shared})

    res = run_bass_kernel_spmd(nc, in_maps, core_ids=list(range(NCORES)))
    _CACHE["last"] = res
    outs = [res.results[c]["out"] for c in range(NCORES)]   # each [T, V]
    full = np.stack(outs)                                    # [8, 768, V]
    full = full.reshape(NCORES, BL, S, V).transpose(2, 0, 1, 3).reshape(S, B, V)
    full = np.ascontiguousarray(full)
    if np.any(out_b):
        full += out_b.astype(np.float32)
    return full

